# revision 17
# baseline (speedup 1.0000x reference)
"""Trainium2 Bass kernel for nn_BertEncoder_403726926494.

Reference computation (per batch element):
  - ragged sentence extraction from hidden_states, masked-softmax attention
    pooling per sentence with W_doc            -> doc_pooled [B, D, H]
  - query extraction (rows 1..32), masked-softmax pooling with W_query
    broadcast over D                           -> q_bcast   [B, D, H]

Device strategy (SPMD, one program on 8 cores, 8 batch elements per core):
  - Per core-slot, DMA only the used row-span of hidden_states into SBUF
    (slots are assigned from a global sort of spans so the per-slot span is
    a static program constant shared by all cores).
  - Per-token scores s[t] = x_t . W_doc: DVE/GpSimd tensor_tensor multiply
    against a W-broadcast tile, then a free-dim reduce on ACT (activation
    Copy + accum_out) or DVE (tensor_reduce) -- engine choice per slot to
    balance load.
  - softmax without max-subtraction (scores are O(1)):
      alphaU[t,j] = exp(s[t] + logSel[t,j])   one ACT op per chunk, where
    logSel is a host-built {0, -1e30} mask marking token t in sentence j
    (columns padded to 32 with -1e30).
      num[j,:H] | den[j] = alphaU^T @ [X | 1]  PE matmul with a ones-column
    appended to X; 4 slots share one PSUM tile via tile_position col-groups.
  - out[j] = num[j] / (den[j] + eps)  (eps keeps empty sentences at 0).
  - Query path packs 4 examples x 32 query rows onto 128 partitions; the
    query-length mask and example-block structure fold into one host-built
    log-mask. q_pooled is broadcast over D on the host.
  - b_doc / b_query shift every score in a softmax segment equally, so they
    cancel and are ignored.
"""

import numpy as np

B, L, H = 64, 512, 768
D, S, Q = 16, 64, 32
NCORES = 8
SLOTS = 8
MPAD = 32  # selector columns padded to one PE col-group
NEG_BIAS = -1.0e30
DEN_EPS = 1.0e-30

# Engine assignment knobs (tuned from traces):
#   score TT multiply per slot: "dve" or "gps"
#   score reduce per slot: "act" (per-chunk accum) or "dve" (merged reduce)
TT_ENGINE = ["dve"] * SLOTS
RED_ENGINE = ["act", "act", "act", "act", "act", "act", "dve", "dve"]
Q_RED_ENGINE = "act"

_compiled: dict = {}


def _slot_geometry(slot_spans):
    nts = [(sp + 127) // 128 for sp in slot_spans]
    rems = [sp - 128 * (nt - 1) for sp, nt in zip(slot_spans, nts)]
    coffs = [0]
    for nt in nts:
        coffs.append(coffs[-1] + nt)
    return nts, rems, coffs


def _build(slot_spans):
    """Build + compile the SPMD Bass program for the given per-slot spans."""
    from contextlib import ExitStack

    import concourse.bacc as bacc
    import concourse.tile as tile
    from concourse import mybir

    f32 = mybir.dt.float32
    MULT = mybir.AluOpType.mult
    ADD = mybir.AluOpType.add
    EXP = mybir.ActivationFunctionType.Exp
    COPY = mybir.ActivationFunctionType.Copy

    nts, rems, coffs = _slot_geometry(slot_spans)
    ntsum = coffs[-1]

    nc = bacc.Bacc(
        "TRN2", target_bir_lowering=False, debug=False, num_devices=NCORES
    )
    hidden = nc.dram_tensor("hidden", [SLOTS, L, H], f32, kind="ExternalInput").ap()
    wd = nc.dram_tensor("wd", [1, H], f32, kind="ExternalInput").ap()
    wq = nc.dram_tensor("wq", [1, H], f32, kind="ExternalInput").ap()
    selt = nc.dram_tensor(
        "selt", [128, ntsum, MPAD], f32, kind="ExternalInput"
    ).ap()
    qmask = nc.dram_tensor("qmask", [128, 2, MPAD], f32, kind="ExternalInput").ap()
    doc_out = nc.dram_tensor("doc_out", [SLOTS, D, H], f32, kind="ExternalOutput").ap()
    q_out = nc.dram_tensor("q_out", [SLOTS, H], f32, kind="ExternalOutput").ap()

    with tile.TileContext(nc) as tc, ExitStack() as ctx:
        const = ctx.enter_context(tc.tile_pool(name="const", bufs=1))

        wrow_d = const.tile([1, H], f32)
        nc.sync.dma_start(out=wrow_d[:], in_=wd[:])
        wrow_q = const.tile([1, H], f32)
        nc.sync.dma_start(out=wrow_q[:], in_=wq[:])
        selt_t = const.tile([128, ntsum, MPAD], f32)
        nc.sync.dma_start(out=selt_t[:], in_=selt[:])
        qmask_t = const.tile([128, 2, MPAD], f32)
        nc.sync.dma_start(out=qmask_t[:], in_=qmask[:])

        # Broadcast W rows across all 128 partitions (gpsimd custom op).
        wb_d = const.tile([128, H], f32)
        wb_q = const.tile([128, H], f32)
        nc.gpsimd.partition_broadcast(wb_d[:], wrow_d[:])
        nc.gpsimd.partition_broadcast(wb_q[:], wrow_q[:])

        xpool = ctx.enter_context(tc.tile_pool(name="xp", bufs=4))
        apool = ctx.enter_context(tc.tile_pool(name="apl", bufs=4))
        scrp = ctx.enter_context(tc.tile_pool(name="scr", bufs=2))
        outp = ctx.enter_context(tc.tile_pool(name="outp", bufs=2))
        smallp = ctx.enter_context(tc.tile_pool(name="smallp", bufs=4))
        qpoolp = ctx.enter_context(tc.tile_pool(name="qpl", bufs=2))
        nump = ctx.enter_context(tc.tile_pool(name="nump", bufs=2, space="PSUM"))
        qnump = ctx.enter_context(tc.tile_pool(name="qnump", bufs=1, space="PSUM"))

        # ---- scores: xw = x * W_bcast (TT), then free-dim reduce -> scol ----
        def emit_scores(x_ap_full, nt, rem, scol, wb, name, tt_eng, red_eng):
            # x_ap_full: [128, nt, H(+1)] view; uses cols 0:H
            xw = scrp.tile([128, nt, H], f32, tag="scratch", name=f"xw{name}")
            tt = nc.gpsimd if tt_eng == "gps" else nc.vector
            if nt > 1:
                tt.tensor_tensor(
                    out=xw[:, 0 : nt - 1, :],
                    in0=x_ap_full[:, 0 : nt - 1, 0:H],
                    in1=wb[:].rearrange("p (o h) -> p o h", o=1).broadcast_to(
                        [128, nt - 1, H]
                    ),
                    op=MULT,
                )
            tt.tensor_tensor(
                out=xw[0:rem, nt - 1, :],
                in0=x_ap_full[0:rem, nt - 1, 0:H],
                in1=wb[0:rem, :],
                op=MULT,
            )
            if red_eng == "dve":
                if nt > 1:
                    nc.vector.tensor_reduce(
                        out=scol[:, 0 : nt - 1],
                        in_=xw[:, 0 : nt - 1, :],
                        axis=mybir.AxisListType.X,
                        op=ADD,
                    )
                nc.vector.tensor_reduce(
                    out=scol[0:rem, nt - 1 : nt],
                    in_=xw[0:rem, nt - 1, :],
                    axis=mybir.AxisListType.X,
                    op=ADD,
                )
            else:
                s2 = scrp.tile([128, H], f32, tag="scratch2", name=f"s2{name}")
                for c in range(nt):
                    cnt = 128 if c < nt - 1 else rem
                    nc.scalar.activation(
                        s2[0:cnt, :], xw[0:cnt, c, :], COPY,
                        bias=0.0, scale=1.0,
                        accum_out=scol[0:cnt, c : c + 1],
                    )

        # ---- doc slots: per-slot pipeline; two groups of 4 share PSUM tiles
        # via PE col-groups. Slots are emitted alternating between the two
        # groups so independent work overlaps and consecutive slots' matmuls
        # land on different col-groups (concurrent PE streams).
        numgs = {}

        def emit_slot(s):
            g, k = divmod(s, 4)
            if g not in numgs:
                numgs[g] = nump.tile([128, 1024], f32, tag="num", name=f"num{g}")
            numg = numgs[g]
            nt, rem, coff = nts[s], rems[s], coffs[s]
            x = xpool.tile([128, nt, H + 1], f32, tag="x", name=f"x{s}")
            ldeng = nc.sync if s % 2 == 0 else nc.scalar
            if nt > 1:
                ldeng.dma_start(
                    out=x[:, 0 : nt - 1, 0:H],
                    in_=hidden[s, 0 : (nt - 1) * 128, :].rearrange(
                        "(c p) h -> p c h", p=128
                    ),
                )
            ldeng.dma_start(
                out=x[0:rem, nt - 1, 0:H],
                in_=hidden[s, (nt - 1) * 128 : (nt - 1) * 128 + rem, :],
            )
            nc.vector.memset(x[:, :, H : H + 1], 1.0)

            scol = smallp.tile([128, nt], f32, tag="scol", name=f"scol{s}")
            emit_scores(
                x[:], nt, rem, scol, wb_d, f"d{s}", TT_ENGINE[s], RED_ENGINE[s]
            )

            at = apool.tile([128, nt, MPAD], f32, tag="at", name=f"at{s}")
            for c in range(nt):
                cnt = 128 if c < nt - 1 else rem
                nc.scalar.activation(
                    at[0:cnt, c, :],
                    selt_t[0:cnt, coff + c, :],
                    EXP,
                    bias=scol[0:cnt, c : c + 1],
                    scale=1.0,
                )
            for c in range(nt):
                cnt = 128 if c < nt - 1 else rem
                first, last = c == 0, c == nt - 1
                nc.tensor.matmul(
                    numg[32 * k : 32 * k + MPAD, 0:512],
                    at[0:cnt, c, :],
                    x[0:cnt, c, 0:512],
                    start=first, stop=last,
                    tile_position=(0, 32 * k),
                    skip_group_check=True,
                )
                nc.tensor.matmul(
                    numg[32 * k : 32 * k + MPAD, 512 : H + 1],
                    at[0:cnt, c, :],
                    x[0:cnt, c, 512 : H + 1],
                    start=first, stop=last,
                    tile_position=(0, 32 * k),
                    skip_group_check=True,
                )

        def finish_group(g):
            numg = numgs[g]
            de = smallp.tile([128, 1], f32, tag="de", name=f"de{g}")
            nc.vector.tensor_scalar(
                out=de[:], in0=numg[:, H : H + 1], scalar1=DEN_EPS,
                scalar2=None, op0=ADD,
            )
            rec = smallp.tile([128, 1], f32, tag="rec", name=f"rec{g}")
            nc.vector.reciprocal(rec[:], de[:])
            do = outp.tile([128, H], f32, tag="do", name=f"do{g}")
            nc.scalar.activation(
                do[:], numg[:, 0:H], COPY, bias=0.0, scale=rec[:, 0:1]
            )
            for k in range(4):
                nc.scalar.dma_start(
                    out=doc_out[4 * g + k, :, :],
                    in_=do[32 * k : 32 * k + D, :],
                )

        # ---- query: two batches of 4 examples x 32 rows -> one PSUM tile ----
        def emit_query(qnumg, b):
            qpack = qpoolp.tile([128, H + 1], f32, tag="qpack", name=f"qpack{b}")
            for sub in range(4):
                nc.gpsimd.dma_start(
                    out=qpack[32 * sub : 32 * sub + 32, 0:H],
                    in_=hidden[4 * b + sub, 1 : 1 + Q, :],
                )
            nc.vector.memset(qpack[:, H : H + 1], 1.0)
            qscol = smallp.tile([128, 1], f32, tag="qscol", name=f"qscol{b}")
            emit_scores(
                qpack[:].rearrange("p (o h) -> p o h", o=1), 1, 128, qscol, wb_q,
                f"q{b}", "dve", Q_RED_ENGINE,
            )
            qat = apool.tile([128, MPAD], f32, tag="qat", name=f"qat{b}")
            nc.scalar.activation(
                qat[:], qmask_t[:, b, :], EXP, bias=qscol[:, 0:1], scale=1.0
            )
            nc.tensor.matmul(
                qnumg[32 * b : 32 * b + MPAD, 0:512],
                qat[:], qpack[:, 0:512],
                start=True, stop=True, tile_position=(0, 32 * b),
            )
            nc.tensor.matmul(
                qnumg[32 * b : 32 * b + MPAD, 512 : H + 1],
                qat[:], qpack[:, 512 : H + 1],
                start=True, stop=True, tile_position=(0, 32 * b),
            )

        qnumg = qnump.tile([64, 1024], f32, tag="qnum", name="qnum")
        for s in (0, 4, 1, 5):
            emit_slot(s)
        emit_query(qnumg, 0)
        for s in (2, 6, 3, 7):
            emit_slot(s)
        emit_query(qnumg, 1)
        finish_group(0)
        finish_group(1)

        qde = smallp.tile([64, 1], f32, tag="qde", name="qde")
        nc.vector.tensor_scalar(
            out=qde[:], in0=qnumg[:, H : H + 1], scalar1=DEN_EPS,
            scalar2=None, op0=ADD,
        )
        qrec = smallp.tile([64, 1], f32, tag="qrec", name="qrec")
        nc.vector.reciprocal(qrec[:], qde[:])
        qo = outp.tile([64, H], f32, tag="qo", name="qo")
        nc.scalar.activation(
            qo[:], qnumg[:, 0:H], COPY, bias=0.0, scale=qrec[:, 0:1]
        )
        for b in range(2):
            nc.sync.dma_start(
                out=q_out[4 * b : 4 * b + 4, :],
                in_=qo[32 * b : 32 * b + 4, :],
            )

    nc.compile()
    return nc


def _prepare(query_len, seq_lens):
    """Host-side geometry: spans, slot assignment, selector/mask arrays."""
    ql = np.asarray(query_len).astype(np.int64)
    sl = np.asarray(seq_lens).astype(np.int64)
    offs = ql[:, None] + 2 + np.cumsum(sl, axis=1) - sl  # [B, D] sentence starts
    end = ql + 2 + sl.sum(axis=1)
    span = np.maximum(end, 1 + Q)  # query rows 1..32 must be covered
    order = np.argsort(-span, kind="stable")  # rank -> example id
    slot_spans = tuple(int(span[order[8 * s]]) for s in range(SLOTS))
    nts, rems, coffs = _slot_geometry(slot_spans)
    ntsum = coffs[-1]

    selt_all = np.full((NCORES, 128, ntsum, MPAD), NEG_BIAS, np.float32)
    qmask_all = np.full((NCORES, 128, 2, MPAD), NEG_BIAS, np.float32)
    ex_map = np.empty((NCORES, SLOTS), np.int64)
    for c in range(NCORES):
        for s in range(SLOTS):
            e = int(order[8 * s + c])
            ex_map[c, s] = e
            for j in range(D):
                ln = int(sl[e, j])
                if ln == 0:
                    continue
                o = int(offs[e, j])
                t = np.arange(o, o + ln)
                selt_all[c, t % 128, coffs[s] + t // 128, j] = 0.0
            b, sub = divmod(s, 4)
            qmask_all[c, 32 * sub : 32 * sub + int(ql[e]), b, sub] = 0.0
    return slot_spans, ex_map, selt_all, qmask_all


def kernel(hidden_states, W_doc, b_doc, W_query, b_query, query_len, seq_lens):
    hs = np.ascontiguousarray(np.asarray(hidden_states, dtype=np.float32))
    wd = np.ascontiguousarray(np.asarray(W_doc, np.float32).reshape(1, H))
    wq = np.ascontiguousarray(np.asarray(W_query, np.float32).reshape(1, H))

    slot_spans, ex_map, selt_all, qmask_all = _prepare(query_len, seq_lens)

    nc = _compiled.get(slot_spans)
    if nc is None:
        nc = _build(slot_spans)
        _compiled[slot_spans] = nc

    in_maps = []
    for c in range(NCORES):
        in_maps.append(
            {
                "hidden": np.ascontiguousarray(hs[ex_map[c]]),
                "wd": wd,
                "wq": wq,
                "selt": selt_all[c],
                "qmask": qmask_all[c],
            }
        )

    from concourse.bass_utils import run_bass_kernel_spmd

    res = run_bass_kernel_spmd(nc, in_maps, list(range(NCORES)))

    doc = np.empty((B, D, H), np.float32)
    qp = np.empty((B, H), np.float32)
    for c in range(NCORES):
        r = res.results[c]
        for s in range(SLOTS):
            e = int(ex_map[c, s])
            doc[e] = r["doc_out"][s]
            qp[e] = r["q_out"][s]
    q_bcast = np.broadcast_to(qp[:, None, :], (B, D, H))
    return doc, q_bcast


# revision 19
# speedup vs baseline: 1.1276x; 1.1276x over previous
"""Trainium2 Bass kernel for nn_BertEncoder_403726926494.

Reference computation (per batch element):
  - ragged sentence extraction from hidden_states, masked-softmax attention
    pooling per sentence with W_doc            -> doc_pooled [B, D, H]
  - query extraction (rows 1..32), masked-softmax pooling with W_query
    broadcast over D                           -> q_bcast   [B, D, H]

Device strategy (SPMD, one program on 8 cores, 8 batch elements per core):
  - Per core-slot, DMA only the used row-span of hidden_states into SBUF
    (slots are assigned from a global sort of spans so the per-slot span is
    a static program constant shared by all cores).
  - Per-token scores s[t] = x_t . W_doc: DVE/GpSimd tensor_tensor multiply
    against a W-broadcast tile, then a free-dim reduce on ACT (activation
    Copy + accum_out) or DVE (tensor_reduce) -- engine choice per slot to
    balance load.
  - softmax without max-subtraction (scores are O(1)):
      alphaU[t,j] = exp(s[t] + logSel[t,j])   one ACT op per chunk, where
    logSel is a host-built {0, -1e30} mask marking token t in sentence j
    (columns padded to 32 with -1e30).
      num[j,:H] | den[j] = alphaU^T @ [X | 1]  PE matmul with a ones-column
    appended to X; 4 slots share one PSUM tile via tile_position col-groups.
  - out[j] = num[j] / (den[j] + eps)  (eps keeps empty sentences at 0).
  - Query path packs 4 examples x 32 query rows onto 128 partitions; the
    query-length mask and example-block structure fold into one host-built
    log-mask. q_pooled is broadcast over D on the host.
  - b_doc / b_query shift every score in a softmax segment equally, so they
    cancel and are ignored.
"""

import numpy as np

B, L, H = 64, 512, 768
D, S, Q = 16, 64, 32
NCORES = 8
SLOTS = 8
MPAD = 32  # selector columns padded to one PE col-group
NEG_BIAS = -1.0e30
DEN_EPS = 1.0e-30

# Engine assignment knobs (tuned from traces):
#   score TT multiply per slot: "dve" or "gps"
#   score reduce per slot: "act" (per-chunk accum) or "dve" (merged reduce)
TT_ENGINE = ["dve"] * SLOTS
RED_ENGINE = ["act", "act", "act", "act", "act", "act", "dve", "dve"]
Q_RED_ENGINE = "act"

_compiled: dict = {}


def _slot_geometry(slot_spans):
    nts = [(sp + 127) // 128 for sp in slot_spans]
    rems = [sp - 128 * (nt - 1) for sp, nt in zip(slot_spans, nts)]
    coffs = [0]
    for nt in nts:
        coffs.append(coffs[-1] + nt)
    return nts, rems, coffs


def _build(slot_spans):
    """Build + compile the SPMD Bass program for the given per-slot spans."""
    from contextlib import ExitStack

    import concourse.bacc as bacc
    import concourse.tile as tile
    from concourse import mybir

    f32 = mybir.dt.float32
    MULT = mybir.AluOpType.mult
    ADD = mybir.AluOpType.add
    EXP = mybir.ActivationFunctionType.Exp
    COPY = mybir.ActivationFunctionType.Copy

    nts, rems, coffs = _slot_geometry(slot_spans)
    ntsum = coffs[-1]
    foffs = [0]
    for nt in nts:
        foffs.append(foffs[-1] + nt - 1)
    roffs = [0]
    for r in rems:
        roffs.append(roffs[-1] + r)

    nc = bacc.Bacc(
        "TRN2", target_bir_lowering=False, debug=False, num_devices=NCORES
    )
    nfull = sum(nt - 1 for nt in nts)
    nremtot = sum(rems)
    sfull = nc.dram_tensor(
        "sfull", [128, max(nfull, 1), H], f32, kind="ExternalInput"
    ).ap()
    srem = nc.dram_tensor("srem", [nremtot, H], f32, kind="ExternalInput").ap()
    qstage = nc.dram_tensor("qstage", [2, 128, H], f32, kind="ExternalInput").ap()
    wd = nc.dram_tensor("wd", [1, H], f32, kind="ExternalInput").ap()
    wq = nc.dram_tensor("wq", [1, H], f32, kind="ExternalInput").ap()
    selt = nc.dram_tensor(
        "selt", [128, ntsum, MPAD], f32, kind="ExternalInput"
    ).ap()
    qmask = nc.dram_tensor("qmask", [128, 2, MPAD], f32, kind="ExternalInput").ap()
    doc_out = nc.dram_tensor("doc_out", [SLOTS, D, H], f32, kind="ExternalOutput").ap()
    q_out = nc.dram_tensor("q_out", [SLOTS, H], f32, kind="ExternalOutput").ap()

    with tile.TileContext(nc) as tc, ExitStack() as ctx:
        const = ctx.enter_context(tc.tile_pool(name="const", bufs=1))

        wrow_d = const.tile([1, H], f32)
        nc.sync.dma_start(out=wrow_d[:], in_=wd[:])
        wrow_q = const.tile([1, H], f32)
        nc.sync.dma_start(out=wrow_q[:], in_=wq[:])
        selt_t = const.tile([128, ntsum, MPAD], f32)
        nc.sync.dma_start(out=selt_t[:], in_=selt[:])
        qmask_t = const.tile([128, 2, MPAD], f32)
        nc.sync.dma_start(out=qmask_t[:], in_=qmask[:])

        # Broadcast W rows across all 128 partitions (gpsimd custom op).
        wb_d = const.tile([128, H], f32)
        wb_q = const.tile([128, H], f32)
        nc.gpsimd.partition_broadcast(wb_d[:], wrow_d[:])
        nc.gpsimd.partition_broadcast(wb_q[:], wrow_q[:])

        xpool = ctx.enter_context(tc.tile_pool(name="xp", bufs=4))
        apool = ctx.enter_context(tc.tile_pool(name="apl", bufs=4))
        scrp = ctx.enter_context(tc.tile_pool(name="scr", bufs=2))
        outp = ctx.enter_context(tc.tile_pool(name="outp", bufs=2))
        smallp = ctx.enter_context(tc.tile_pool(name="smallp", bufs=4))
        qpoolp = ctx.enter_context(tc.tile_pool(name="qpl", bufs=2))
        nump = ctx.enter_context(tc.tile_pool(name="nump", bufs=2, space="PSUM"))
        qnump = ctx.enter_context(tc.tile_pool(name="qnump", bufs=1, space="PSUM"))

        # ---- scores: xw = x * W_bcast (TT), then free-dim reduce -> scol ----
        def emit_scores(x_ap_full, nt, rem, scol, wb, name, tt_eng, red_eng):
            # x_ap_full: [128, nt, H(+1)] view; uses cols 0:H
            xw = scrp.tile([128, nt, H], f32, tag="scratch", name=f"xw{name}")
            tt = nc.gpsimd if tt_eng == "gps" else nc.vector
            if nt > 1:
                tt.tensor_tensor(
                    out=xw[:, 0 : nt - 1, :],
                    in0=x_ap_full[:, 0 : nt - 1, 0:H],
                    in1=wb[:].rearrange("p (o h) -> p o h", o=1).broadcast_to(
                        [128, nt - 1, H]
                    ),
                    op=MULT,
                )
            tt.tensor_tensor(
                out=xw[0:rem, nt - 1, :],
                in0=x_ap_full[0:rem, nt - 1, 0:H],
                in1=wb[0:rem, :],
                op=MULT,
            )
            if red_eng == "dve":
                if nt > 1:
                    nc.vector.tensor_reduce(
                        out=scol[:, 0 : nt - 1],
                        in_=xw[:, 0 : nt - 1, :],
                        axis=mybir.AxisListType.X,
                        op=ADD,
                    )
                nc.vector.tensor_reduce(
                    out=scol[0:rem, nt - 1 : nt],
                    in_=xw[0:rem, nt - 1, :],
                    axis=mybir.AxisListType.X,
                    op=ADD,
                )
            else:
                s2 = scrp.tile([128, H], f32, tag="scratch2", name=f"s2{name}")
                for c in range(nt):
                    cnt = 128 if c < nt - 1 else rem
                    nc.scalar.activation(
                        s2[0:cnt, :], xw[0:cnt, c, :], COPY,
                        bias=0.0, scale=1.0,
                        accum_out=scol[0:cnt, c : c + 1],
                    )

        # ---- doc slots: per-slot pipeline; two groups of 4 share PSUM tiles
        # via PE col-groups. Slots are emitted alternating between the two
        # groups so independent work overlaps and consecutive slots' matmuls
        # land on different col-groups (concurrent PE streams).
        numgs = {}

        def emit_slot(s):
            g, k = divmod(s, 4)
            if g not in numgs:
                numgs[g] = nump.tile([128, 1024], f32, tag="num", name=f"num{g}")
            numg = numgs[g]
            nt, rem, coff = nts[s], rems[s], coffs[s]
            x = xpool.tile([128, nt, H + 1], f32, tag="x", name=f"x{s}")
            if nt > 1:
                nc.sync.dma_start(
                    out=x[:, 0 : nt - 1, 0:H],
                    in_=sfull[:, foffs[s] : foffs[s] + nt - 1, :],
                )
            nc.sync.dma_start(
                out=x[0:rem, nt - 1, 0:H],
                in_=srem[roffs[s] : roffs[s] + rem, :],
            )
            nc.vector.memset(x[:, :, H : H + 1], 1.0)

            scol = smallp.tile([128, nt], f32, tag="scol", name=f"scol{s}")
            emit_scores(
                x[:], nt, rem, scol, wb_d, f"d{s}", TT_ENGINE[s], RED_ENGINE[s]
            )

            at = apool.tile([128, nt, MPAD], f32, tag="at", name=f"at{s}")
            for c in range(nt):
                cnt = 128 if c < nt - 1 else rem
                nc.scalar.activation(
                    at[0:cnt, c, :],
                    selt_t[0:cnt, coff + c, :],
                    EXP,
                    bias=scol[0:cnt, c : c + 1],
                    scale=1.0,
                )
            for c in range(nt):
                cnt = 128 if c < nt - 1 else rem
                first, last = c == 0, c == nt - 1
                nc.tensor.matmul(
                    numg[32 * k : 32 * k + MPAD, 0:512],
                    at[0:cnt, c, :],
                    x[0:cnt, c, 0:512],
                    start=first, stop=last,
                    tile_position=(0, 32 * k),
                    skip_group_check=True,
                )
                nc.tensor.matmul(
                    numg[32 * k : 32 * k + MPAD, 512 : H + 1],
                    at[0:cnt, c, :],
                    x[0:cnt, c, 512 : H + 1],
                    start=first, stop=last,
                    tile_position=(0, 32 * k),
                    skip_group_check=True,
                )

        def finish_group(g):
            numg = numgs[g]
            de = smallp.tile([128, 1], f32, tag="de", name=f"de{g}")
            nc.vector.tensor_scalar(
                out=de[:], in0=numg[:, H : H + 1], scalar1=DEN_EPS,
                scalar2=None, op0=ADD,
            )
            rec = smallp.tile([128, 1], f32, tag="rec", name=f"rec{g}")
            nc.vector.reciprocal(rec[:], de[:])
            do = outp.tile([128, H], f32, tag="do", name=f"do{g}")
            nc.scalar.activation(
                do[:], numg[:, 0:H], COPY, bias=0.0, scale=rec[:, 0:1]
            )
            for k in range(4):
                nc.scalar.dma_start(
                    out=doc_out[4 * g + k, :, :],
                    in_=do[32 * k : 32 * k + D, :],
                )

        # ---- query: two batches of 4 examples x 32 rows -> one PSUM tile ----
        def emit_query(qnumg, b):
            qpack = qpoolp.tile([128, H + 1], f32, tag="qpack", name=f"qpack{b}")
            nc.sync.dma_start(out=qpack[:, 0:H], in_=qstage[b, :, :])
            nc.vector.memset(qpack[:, H : H + 1], 1.0)
            qscol = smallp.tile([128, 1], f32, tag="qscol", name=f"qscol{b}")
            emit_scores(
                qpack[:].rearrange("p (o h) -> p o h", o=1), 1, 128, qscol, wb_q,
                f"q{b}", "dve", Q_RED_ENGINE,
            )
            qat = apool.tile([128, MPAD], f32, tag="qat", name=f"qat{b}")
            nc.scalar.activation(
                qat[:], qmask_t[:, b, :], EXP, bias=qscol[:, 0:1], scale=1.0
            )
            nc.tensor.matmul(
                qnumg[32 * b : 32 * b + MPAD, 0:512],
                qat[:], qpack[:, 0:512],
                start=True, stop=True, tile_position=(0, 32 * b),
            )
            nc.tensor.matmul(
                qnumg[32 * b : 32 * b + MPAD, 512 : H + 1],
                qat[:], qpack[:, 512 : H + 1],
                start=True, stop=True, tile_position=(0, 32 * b),
            )

        qnumg = qnump.tile([64, 1024], f32, tag="qnum", name="qnum")
        for s in (0, 4, 1, 5):
            emit_slot(s)
        emit_query(qnumg, 0)
        for s in (2, 6, 3, 7):
            emit_slot(s)
        emit_query(qnumg, 1)
        finish_group(0)
        finish_group(1)

        qde = smallp.tile([64, 1], f32, tag="qde", name="qde")
        nc.vector.tensor_scalar(
            out=qde[:], in0=qnumg[:, H : H + 1], scalar1=DEN_EPS,
            scalar2=None, op0=ADD,
        )
        qrec = smallp.tile([64, 1], f32, tag="qrec", name="qrec")
        nc.vector.reciprocal(qrec[:], qde[:])
        qo = outp.tile([64, H], f32, tag="qo", name="qo")
        nc.scalar.activation(
            qo[:], qnumg[:, 0:H], COPY, bias=0.0, scale=qrec[:, 0:1]
        )
        for b in range(2):
            nc.sync.dma_start(
                out=q_out[4 * b : 4 * b + 4, :],
                in_=qo[32 * b : 32 * b + 4, :],
            )

    nc.compile()
    return nc


def _prepare(query_len, seq_lens):
    """Host-side geometry: spans, slot assignment, selector/mask arrays."""
    ql = np.asarray(query_len).astype(np.int64)
    sl = np.asarray(seq_lens).astype(np.int64)
    offs = ql[:, None] + 2 + np.cumsum(sl, axis=1) - sl  # [B, D] sentence starts
    end = ql + 2 + sl.sum(axis=1)
    span = np.maximum(end, 1 + Q)  # query rows 1..32 must be covered
    order = np.argsort(-span, kind="stable")  # rank -> example id
    slot_spans = tuple(int(span[order[8 * s]]) for s in range(SLOTS))
    nts, rems, coffs = _slot_geometry(slot_spans)
    ntsum = coffs[-1]

    selt_all = np.full((NCORES, 128, ntsum, MPAD), NEG_BIAS, np.float32)
    qmask_all = np.full((NCORES, 128, 2, MPAD), NEG_BIAS, np.float32)
    ex_map = np.empty((NCORES, SLOTS), np.int64)
    for c in range(NCORES):
        for s in range(SLOTS):
            e = int(order[8 * s + c])
            ex_map[c, s] = e
            for j in range(D):
                ln = int(sl[e, j])
                if ln == 0:
                    continue
                o = int(offs[e, j])
                t = np.arange(o, o + ln)
                selt_all[c, t % 128, coffs[s] + t // 128, j] = 0.0
            b, sub = divmod(s, 4)
            qmask_all[c, 32 * sub : 32 * sub + int(ql[e]), b, sub] = 0.0
    return slot_spans, ex_map, selt_all, qmask_all


def kernel(hidden_states, W_doc, b_doc, W_query, b_query, query_len, seq_lens):
    hs = np.ascontiguousarray(np.asarray(hidden_states, dtype=np.float32))
    wd = np.ascontiguousarray(np.asarray(W_doc, np.float32).reshape(1, H))
    wq = np.ascontiguousarray(np.asarray(W_query, np.float32).reshape(1, H))

    slot_spans, ex_map, selt_all, qmask_all = _prepare(query_len, seq_lens)

    nc = _compiled.get(slot_spans)
    if nc is None:
        nc = _build(slot_spans)
        _compiled[slot_spans] = nc

    nts, rems, _ = _slot_geometry(slot_spans)
    nfull = sum(nt - 1 for nt in nts)
    nremtot = sum(rems)

    in_maps = []
    for c in range(NCORES):
        sfull = np.empty((128, max(nfull, 1), H), np.float32)
        srem = np.empty((nremtot, H), np.float32)
        qstage = np.empty((2, 128, H), np.float32)
        fo = ro = 0
        for s in range(SLOTS):
            e = int(ex_map[c, s])
            nt, rem = nts[s], rems[s]
            if nt > 1:
                sfull[:, fo : fo + nt - 1, :] = (
                    hs[e, 0 : (nt - 1) * 128, :]
                    .reshape(nt - 1, 128, H)
                    .transpose(1, 0, 2)
                )
                fo += nt - 1
            srem[ro : ro + rem] = hs[e, (nt - 1) * 128 : (nt - 1) * 128 + rem, :]
            ro += rem
            b, sub = divmod(s, 4)
            qstage[b, 32 * sub : 32 * sub + 32, :] = hs[e, 1 : 1 + Q, :]
        in_maps.append(
            {
                "sfull": sfull,
                "srem": srem,
                "qstage": qstage,
                "wd": wd,
                "wq": wq,
                "selt": selt_all[c],
                "qmask": qmask_all[c],
            }
        )

    from concourse.bass_utils import run_bass_kernel_spmd

    res = run_bass_kernel_spmd(nc, in_maps, list(range(NCORES)))

    doc = np.empty((B, D, H), np.float32)
    qp = np.empty((B, H), np.float32)
    for c in range(NCORES):
        r = res.results[c]
        for s in range(SLOTS):
            e = int(ex_map[c, s])
            doc[e] = r["doc_out"][s]
            qp[e] = r["q_out"][s]
    q_bcast = np.broadcast_to(qp[:, None, :], (B, D, H))
    return doc, q_bcast
